# revision 11
# baseline (speedup 1.0000x reference)
"""CapsuleLayer dynamic-routing kernel for 8 Trainium2 NeuronCores.

Strategy: data-parallel over batch (64 / 8 cores = 8 batches per core, no
collectives). Inside each core:
  - u_hat[b,i,j,e] = sum_d u[b,i,d] W[i,j,d,e] is built once via TensorE
    matmuls with a host-prepacked block-diagonal u operand, and kept
    SBUF-resident in bf16 with partition layout (b4, i32) per "quad"
    (quad q holds batches 4q..4q+3), free layout (quad, iblk, j*16+e).
  - 3 routing iterations. Weighted sum s via 4x row+col tile_position
    packed matmuls (contraction over the 32 i's in each partition group),
    diag-extraction via a fixed mask + selector matmul. Agreement update
    via DVE multiply + grouped reduce. Softmax via ACT exp + DVE reduce.
"""

import sys

sys.path.insert(0, "/opt/trn_rl_repo")

import numpy as np
import ml_dtypes

B, NI, DI, NO, DO = 64, 2048, 8, 32, 16
NC_CORES = 8
BL = B // NC_CORES          # 8 batches per core
JE = NO * DO                # 512
NBLK = NI // 32             # 64 blocks of 32 input capsules
NQ = 2                      # 2 quads of 4 batches
EPS = 1e-7
BF16 = ml_dtypes.bfloat16

_cache = {}


def _build_program():
    import concourse.bass as bass
    import concourse.bacc as bacc
    import concourse.mybir as mybir
    import concourse.tile as tile

    f32 = mybir.dt.float32
    bf16 = mybir.dt.bfloat16

    nc = bacc.Bacc("TRN2", target_bir_lowering=False, debug=False,
                   num_devices=NC_CORES)

    # DRAM I/O (per core)
    w_d = nc.dram_tensor("w32", [NBLK, 128, 2 * JE], bf16, kind="ExternalInput")
    ubd_d = nc.dram_tensor("ubd32", [NBLK, NQ, 128, 256], bf16,
                           kind="ExternalInput")
    dm_d = nc.dram_tensor("diagmask", [128, JE], f32, kind="ExternalInput")
    sq_d = nc.dram_tensor("selq", [128, NQ, BL], f32, kind="ExternalInput")
    vout_d = nc.dram_tensor("v_out", [BL, JE], f32, kind="ExternalOutput")

    with tile.TileContext(nc) as tc:
        with (
            tc.tile_pool(name="singles", bufs=1) as singles,
            tc.tile_pool(name="wstream", bufs=3) as wpool,
            tc.tile_pool(name="ustream", bufs=3) as upool,
            tc.tile_pool(name="dbuf", bufs=2) as dpool,
            tc.tile_pool(name="small", bufs=2) as spool,
            tc.tile_pool(name="build_ps", bufs=2, space="PSUM") as build_ps,
            tc.tile_pool(name="spass_ps", bufs=2, space="PSUM") as spass_ps,
            tc.tile_pool(name="s_ps", bufs=1, space="PSUM") as s_ps_pool,
        ):
            # ---- persistent SBUF state ----
            UH = singles.tile([128, NQ, NBLK, JE], bf16)      # 128 KiB/part
            LOG = singles.tile([128, NQ, NBLK, NO], f32)      # 16 KiB/part
            C = singles.tile([128, NQ, NBLK, NO], bf16)       # 8 KiB/part
            # masked-c matmul operand: CM[p, blk, (b',j')] = c[p, blk, j']
            # iff b' == p//32 (block-diagonal in the 32-partition ranges)
            CM = singles.tile([128, NBLK, 128], bf16)         # 16 KiB/part
            DM = singles.tile([128, JE], f32)
            SQ = singles.tile([128, NQ, BL], f32)
            VREP = singles.tile([128, NQ, JE], bf16)
            s_sb = singles.tile([BL, JE], f32)
            vb_sb = singles.tile([BL, JE], bf16)

            nc.sync.dma_start(out=DM[:, :], in_=dm_d[:, :])
            nc.sync.dma_start(out=SQ[:, :, :], in_=sq_d[:, :, :])
            nc.vector.memset(LOG[:, :, :, :], 0.0)
            nc.vector.memset(C[:, :, :, :], 1.0 / NO)
            nc.vector.memset(CM[:, :, :], 0.0)

            # ---- phase 1: build u_hat ----
            for blk in range(NBLK):
                w_t = wpool.tile([128, 2 * JE], bf16, tag="w")
                nc.sync.dma_start(out=w_t[:, :], in_=w_d[blk, :, :])
                for q in range(NQ):
                    u_t = upool.tile([128, 256], bf16, tag="u")
                    nc.gpsimd.dma_start(out=u_t[:, :], in_=ubd_d[blk, q, :, :])
                    ps = build_ps.tile([128, JE], f32, tag="bps")
                    nc.tensor.matmul(ps[:, :], u_t[:, 0:128], w_t[:, 0:JE],
                                     start=True, stop=False)
                    nc.tensor.matmul(ps[:, :], u_t[:, 128:256], w_t[:, JE:2 * JE],
                                     start=False, stop=True)
                    # evacuate to bf16 SBUF, alternating engines
                    if (blk * NQ + q) % 2 == 0:
                        nc.vector.tensor_copy(UH[:, q, blk, :], ps[:, :])
                    else:
                        nc.scalar.copy(UH[:, q, blk, :], ps[:, :])

            # ---- routing iterations ----
            CHUNK = 4
            NCHUNK = NBLK // CHUNK

            for t in (1, 2, 3):
                if t > 1:
                    # v (prev iter) -> VREP via replicating DMA
                    for q in range(NQ):
                        vq = vb_sb[4 * q:4 * q + 4, :]
                        src = bass.AP(
                            tensor=vq.tensor,
                            offset=vq.offset,
                            ap=[list(vq.ap[0]), [0, 32], list(vq.ap[1])],
                        )
                        nc.sync.dma_start(out=VREP[:, q, :], in_=src)
                    for q in range(NQ):
                        for ch in range(NCHUNK):
                            blks = slice(ch * CHUNK, (ch + 1) * CHUNK)
                            P = dpool.tile([128, CHUNK, JE], bf16, tag="P")
                            vrb = VREP[:, q, :].unsqueeze(1).broadcast_to(
                                [128, CHUNK, JE])
                            nc.vector.tensor_mul(
                                P[:, :, :], UH[:, q, blks, :], vrb)
                            D = dpool.tile([128, CHUNK, NO], f32, tag="D")
                            nc.vector.tensor_reduce(
                                out=D[:, :, :],
                                in_=P.rearrange("p c (j e) -> p c j e", e=DO),
                                axis=mybir.AxisListType.X,
                                op=mybir.AluOpType.add,
                            )
                            nc.vector.tensor_add(
                                LOG[:, q, blks, :], LOG[:, q, blks, :],
                                D[:, :, :])
                            # softmax on this chunk
                            EX = dpool.tile([128, CHUNK, NO], f32, tag="EX")
                            nc.scalar.activation(
                                EX[:, :, :], LOG[:, q, blks, :],
                                mybir.ActivationFunctionType.Exp)
                            Z = spool.tile([128, CHUNK], f32, tag="Z")
                            nc.vector.tensor_reduce(
                                out=Z[:, :], in_=EX[:, :, :],
                                axis=mybir.AxisListType.X,
                                op=mybir.AluOpType.add,
                            )
                            R = spool.tile([128, CHUNK], f32, tag="R")
                            nc.vector.reciprocal(R[:, :], Z[:, :])
                            rb = R.unsqueeze(2).broadcast_to([128, CHUNK, NO])
                            nc.vector.tensor_mul(
                                C[:, q, blks, :], EX[:, :, :], rb)

                # s-pass: accumulate over all blocks per quad
                s_ps = s_ps_pool.tile([BL, JE], f32, tag="sps")
                for q in range(NQ):
                    # scatter c into the block-diagonal CM operand
                    for bl in range(4):
                        rows = slice(32 * bl, 32 * bl + 32)
                        nc.vector.tensor_copy(
                            CM[rows, :, 32 * bl:32 * bl + 32],
                            C[rows, q, :, :])
                    sp = spass_ps.tile([128, JE], f32, tag="spq")
                    for blk in range(NBLK):
                        nc.tensor.matmul(
                            sp[:, :],
                            CM[:, blk, :],
                            UH[:, q, blk, :],
                            start=(blk == 0), stop=(blk == NBLK - 1),
                        )
                    ME = spool.tile([128, JE], f32, tag="ME")
                    nc.vector.tensor_mul(ME[:, :], sp[:, :], DM[:, :])
                    nc.tensor.matmul(
                        s_ps[:, :], SQ[:, q, :], ME[:, :],
                        start=(q == 0), stop=(q == NQ - 1),
                    )

                # squash
                nc.vector.tensor_copy(s_sb[:, :], s_ps[:, :])
                SQT = spool.tile([BL, JE], f32, tag="SQT")
                nc.vector.tensor_mul(SQT[:, :], s_sb[:, :], s_sb[:, :])
                N2 = spool.tile([BL, NO], f32, tag="N2")
                nc.vector.tensor_reduce(
                    out=N2[:, :],
                    in_=SQT.rearrange("p (j e) -> p j e", e=DO),
                    axis=mybir.AxisListType.X,
                    op=mybir.AluOpType.add,
                )
                NE = spool.tile([BL, NO], f32, tag="NE")
                nc.vector.tensor_scalar_add(NE[:, :], N2[:, :], EPS)
                SRT = spool.tile([BL, NO], f32, tag="SRT")
                nc.scalar.activation(SRT[:, :], NE[:, :],
                                     mybir.ActivationFunctionType.Sqrt)
                T1 = spool.tile([BL, NO], f32, tag="T1")
                nc.vector.tensor_scalar_add(T1[:, :], N2[:, :], 1.0)
                T2 = spool.tile([BL, NO], f32, tag="T2")
                nc.vector.tensor_mul(T2[:, :], T1[:, :], SRT[:, :])
                RC = spool.tile([BL, NO], f32, tag="RC")
                nc.vector.reciprocal(RC[:, :], T2[:, :])
                F = spool.tile([BL, NO], f32, tag="F")
                nc.vector.tensor_mul(F[:, :], N2[:, :], RC[:, :])
                fb = F.unsqueeze(2).broadcast_to([BL, NO, DO])
                if t < 3:
                    nc.vector.tensor_mul(
                        vb_sb.rearrange("p (j e) -> p j e", e=DO),
                        s_sb.rearrange("p (j e) -> p j e", e=DO), fb)
                else:
                    v_sb = singles.tile([BL, JE], f32)
                    nc.vector.tensor_mul(
                        v_sb.rearrange("p (j e) -> p j e", e=DO),
                        s_sb.rearrange("p (j e) -> p j e", e=DO), fb)
                    nc.sync.dma_start(out=vout_d[:, :], in_=v_sb[:, :])

    nc.compile()
    return nc


def _host_prep(u, W):
    """Prepack operands. Returns per-core input maps."""
    # W: [NI, NO, DI, DO] -> w32[blk, h, i*4+dl, j*16+e] = W[blk*32+i, j, 4h+dl, e]
    w32 = (
        W.reshape(NBLK, 32, NO, 2, 4, DO)       # blk, i, j, h, dl, e
        .transpose(0, 1, 4, 3, 2, 5)            # blk, i, dl, h, j, e
        .reshape(NBLK, 128, 2 * JE)
        .astype(BF16)
    )
    # u block-diagonal: ubd[core][blk, h, q, i*4+dl, b*32+i] = u[core*8+4q+b, blk*32+i, 4h+dl]
    ur = u.reshape(NC_CORES, NQ, 4, NBLK, 32, 2, 4)  # c, q, b, blk, i, h, dl
    ubd = np.zeros((NC_CORES, NBLK, NQ, 128, 2, 128), dtype=BF16)
    for i in range(32):
        # target [c, blk, q, dl(4), h(2), b(4)] at rows 4i..4i+4, cols h*128+b*32+i
        blkslice = ur[:, :, :, :, i, :, :].transpose(0, 3, 1, 5, 4, 2)
        ubd[:, :, :, 4 * i:4 * i + 4, :, i::32] = blkslice.astype(BF16)
    ubd = ubd.reshape(NC_CORES, NBLK, NQ, 128, 256)
    # diag mask: dm[p, j*16+e] = (j == p % 32)
    pj = np.arange(128) % 32
    dm = (np.arange(NO)[None, :] == pj[:, None]).astype(np.float32)
    dm = np.repeat(dm, DO, axis=1)  # wrong order? dm[p, j] -> repeat e fast
    dm = dm.reshape(128, NO, DO).reshape(128, JE)
    # selector: sq[q, p, b'] = (b' == 4q + p//32)
    sq = np.zeros((128, NQ, BL), dtype=np.float32)
    for q in range(NQ):
        for p in range(128):
            sq[p, q, 4 * q + p // 32] = 1.0
    return w32, ubd, dm, sq


def kernel(u, W):
    from concourse.bass_utils import run_bass_kernel_spmd

    key = "prog"
    if key not in _cache:
        _cache[key] = _build_program()
    nc = _cache[key]

    w32, ubd, dm, sq = _host_prep(np.asarray(u, np.float32),
                                  np.asarray(W, np.float32))
    in_maps = [
        {"w32": w32, "ubd32": ubd[c], "diagmask": dm, "selq": sq}
        for c in range(NC_CORES)
    ]
    res = run_bass_kernel_spmd(nc, in_maps, list(range(NC_CORES)))
    out = np.concatenate([res.results[c]["v_out"] for c in range(NC_CORES)],
                         axis=0)
    return out.reshape(B, NO, DO).astype(np.float32)
